# revision 42
# baseline (speedup 1.0000x reference)
"""Trainium2 Bass kernel for nn_BlockRecurrentSwinIRBlock (self-contained).

kernel(**inputs) takes the FULL unsharded inputs (B=2048 windows, 64 tokens,
C=256) and returns (output_x, state_out), each (2048, 64, 256) fp32.

Data-parallel over windows across 8 NeuronCores (256 windows/core).

v3 design: flat software-pipelined loop over window-pairs (wp = 128 tokens).
  - QK^T in 4 j-pure PSUM banks (one per 32-lane row group; mixed
    tile_position rows must never share a PSUM bank). E is j-major:
    col = j*512 + (2s+hi)*128 + t*64 + q, so every evacuation is contiguous.
  - exp on ScalarE (4 contiguous [128,512] ops), bias multiply
    E = exp(S)*exp(B) on the otherwise-idle GpSimd engine.
  - static per-purpose PSUM tags (8 banks exactly): qkj0..3 (QK banks,
    shared with 16 proj1/vproj piece tiles per chunk) + o0/o1/d0/d1
    (AV out + softmax sums; the merged out-projection bank shares d0).
  - proj1/vproj for chunk c+1 computed as PE filler interleaved into chunk
    c's attention iterations; fully unrolled loop, bf16 outputs.
"""
import os
import sys
import numpy as np

KDBG = set(os.environ.get("KDBG", "").split(","))

for _p in ("/opt/trn_rl_repo", "/opt/trn_rl_repo/concourse"):
    if _p not in sys.path:
        sys.path.insert(0, _p)

import concourse.bass as bass
import concourse.tile as tile
from concourse import bacc, mybir
from concourse.bass_utils import run_bass_kernel_spmd
from contextlib import ExitStack

N_CORES = 8
NTOK = 64
DIM = 256
B_TOTAL = 2048
W_CORE = B_TOTAL // N_CORES
CH = 4          # window-pairs per chunk

_cache = {}

BF16 = mybir.dt.bfloat16
F8 = mybir.dt.float8e4
F32 = mybir.dt.float32
AF = mybir.ActivationFunctionType
DR = mybir.MatmulPerfMode.DoubleRow
SCALE = (DIM // 8) ** -0.5   # hd^-0.5 = 0.17678
USE_DR = os.environ.get("KDR", "0") == "1"


def build_kernel(nc, tc, W):
    WP = W // 2          # window-pairs per core (128 tokens each)
    T = W * 64
    NCH = WP // CH
    assert WP % CH == 0

    # chunk-major, per-partition-contiguous: one 1KB run per partition per
    # DMA (128 descriptors) instead of the old per-wp strided gather (512).
    xt_s = nc.dram_tensor("xt_s", [WP // CH, 2, 128, CH * 128], BF16,
                          kind="ExternalInput").ap()
    xt_e = nc.dram_tensor("xt_e", [WP // CH, 2, 128, CH * 128], BF16,
                          kind="ExternalInput").ap()
    wkq_s = nc.dram_tensor("wkq_s", [2, 128, 768], BF16, kind="ExternalInput").ap()
    wkq_e = nc.dram_tensor("wkq_e", [2, 128, 768], BF16, kind="ExternalInput").ap()
    if USE_DR:
        xt8_s = nc.dram_tensor("xt8_s", [WP // CH, 2, 128, CH * 128], F8,
                               kind="ExternalInput").ap()
        xt8_e = nc.dram_tensor("xt8_e", [WP // CH, 2, 128, CH * 128], F8,
                               kind="ExternalInput").ap()
        # unscaled fp8 kq weights: state = full k|qv|qh, e = qh only
        wkq8_s = nc.dram_tensor("wkq8_s", [2, 128, 768], F8,
                                kind="ExternalInput").ap()
        wkq8_e = nc.dram_tensor("wkq8_e", [2, 128, 256], F8,
                                kind="ExternalInput").ap()
    wv_s = nc.dram_tensor("wv_s", [2, 128, 256], BF16, kind="ExternalInput").ap()
    wv_e = nc.dram_tensor("wv_e", [2, 128, 256], BF16, kind="ExternalInput").ap()
    wpv = nc.dram_tensor("wpv", [4, 128, 256], BF16, kind="ExternalInput").ap()
    wph = nc.dram_tensor("wph", [4, 128, 256], BF16, kind="ExternalInput").ap()
    # exp(bias) master, j-major: col = j*512 + (2s+hi)*128 + t*64 + q
    ebc = nc.dram_tensor("ebc", [128, 2048], BF16, kind="ExternalInput").ap()

    out_x = nc.dram_tensor("out_x", [T, 256], BF16, kind="ExternalOutput").ap()
    out_h = nc.dram_tensor("out_h", [T, 256], BF16, kind="ExternalOutput").ap()

    xt_d = {0: xt_s, 1: xt_e}
    xt8_d = {0: xt8_s, 1: xt8_e} if USE_DR else {}
    wv_d = {0: wv_s, 1: wv_e}

    with ExitStack() as ctx:
        wpool = ctx.enter_context(tc.tile_pool(name="weights", bufs=1))
        sb = ctx.enter_context(tc.tile_pool(name="sb", bufs=2))
        pp = ctx.enter_context(tc.tile_pool(name="pp", bufs=1, space="PSUM"))

        # ---- weights / constants (resident) ----
        wv = {}
        wkq = {}
        for s, apw in ((0, wkq_s), (1, wkq_e)):
            t_ = wpool.tile([128, 2 * 768], BF16, tag=f"wkq{s}", name=f"wkq{s}t")
            for kk in (0, 1):
                nc.sync.dma_start(t_[:, kk * 768:(kk + 1) * 768], apw[kk])
            wkq[s] = t_
        wkq8 = {}
        if USE_DR:
            w8s = wpool.tile([128, 2 * 768], F8, tag="wkq8s", name="wkq8st")
            for kk in (0, 1):
                nc.sync.dma_start(w8s[:, kk * 768:(kk + 1) * 768], wkq8_s[kk])
            w8e = wpool.tile([128, 2 * 256], F8, tag="wkq8e", name="wkq8et")
            for kk in (0, 1):
                nc.sync.dma_start(w8e[:, kk * 256:(kk + 1) * 256], wkq8_e[kk])
            wkq8 = {0: w8s, 1: w8e}
        for s in (0, 1):
            tv = wpool.tile([128, 2 * 256], BF16, tag=f"wv{s}", name=f"wv{s}t")
            for kk in (0, 1):
                nc.sync.dma_start(tv[:, kk * 256:(kk + 1) * 256], wv_d[s][kk])
            wv[s] = tv
        ebt = wpool.tile([128, 2048], BF16, tag="ebc", name="ebt")
        nc.sync.dma_start(ebt[:], ebc[:])
        wp_t = {}
        for nm, apw in (("x", wpv), ("h", wph)):
            t_ = wpool.tile([128, 4 * 256], BF16, tag=f"wp{nm}", name=f"wp{nm}t")
            for z in range(4):
                nc.sync.dma_start(t_[:, z * 256:(z + 1) * 256], apw[z])
            wp_t[nm] = t_
        ones32 = wpool.tile([128, 32], BF16, tag="ones", name="ones32")
        nc.vector.memset(ones32[:], 1.0)

        # ---- DMA of a chunk's XT (both sides, merged per kk) ----
        def dma_xt(ci):
            XT = {}
            for s in (0, 1):
                t_ = sb.tile([128, 1024], BF16, tag=f"xt{s}", bufs=2,
                             name=f"xt{s}_{ci}")
                nc.sync.dma_start(t_.rearrange("p (k n) -> p k n", k=2),
                                  xt_d[s][ci].rearrange("k p n -> p k n"))
                XT[s] = t_
                if USE_DR:
                    t8 = sb.tile([128, 1024], F8, tag=f"xt8{s}", bufs=2,
                                 name=f"xt8{s}_{ci}")
                    for kk in (0, 1):
                        nc.sync.dma_start(t8[:, kk * 512:(kk + 1) * 512],
                                          xt8_d[s][ci, kk])
                    XT[f"f8_{s}"] = t8
            return XT

        # ---- allocate the kqv tile set for one chunk ----
        def alloc_kqv(ci):
            kqv = {}
            for s in (0, 1):
                kqv[f"yk{s}"] = sb.tile([128, 1024], BF16, tag=f"yk{s}",
                                        bufs=3, name=f"yk{s}_{ci}")
                kqv[f"yq{s}"] = sb.tile([128, 2048], BF16, tag=f"yq{s}",
                                        bufs=3, name=f"yq{s}_{ci}")
                kqv[f"v{s}"] = sb.tile([128, 1024], BF16, tag=f"v{s}",
                                       bufs=3, name=f"v{s}_{ci}")
            return kqv

        # ---- one proj1/vproj piece for chunk ci (16 pieces: 0..15) ----
        # pieces 0..11: proj1 (s, mt); 12..15: vproj (s, wpair)
        # piece p uses psum tag qkj{p%4} ([128,512], 1 bank)
        def piece(pidx, XT, kqv, evict_dve):
            ps = pp.tile([128, 512], F32, tag=f"qkj{pidx % 4}",
                         name=f"pc{pidx}")
            dr_scale = None
            if pidx < 12:
                s, mt = divmod(pidx, 6)
                if USE_DR and (s == 0 or mt >= 4):
                    # fp8 DoubleRow piece with unscaled weights; the softmax
                    # scale is applied at PSUM evacuation.
                    dr_scale = 1.0
                    if mt >= 4:
                        dr_scale = SCALE * SCALE if s == 0 else SCALE
                    elif mt >= 2:
                        dr_scale = SCALE
                    w8 = wkq8[s]
                    mof = mt * 128 if s == 0 else (mt - 4) * 128
                    lhsT = w8.rearrange("p (kk c) -> p kk c", kk=2)[
                        :, :, mof:mof + 128]
                    rhs = XT[f"f8_{s}"].rearrange("p (kk n) -> p kk n", kk=2)
                    nc.tensor.matmul(ps[:], lhsT, rhs, start=True, stop=True,
                                     perf_mode=DR)
                else:
                    for kk in (0, 1):
                        nc.tensor.matmul(
                            ps[:],
                            wkq[s][:, kk * 768 + mt * 128: kk * 768 + (mt + 1) * 128],
                            XT[s][:, kk * 512:(kk + 1) * 512],
                            start=(kk == 0), stop=(kk == 1),
                        )
                if mt < 2:
                    dest = kqv[f"yk{s}"][:, mt * 512:(mt + 1) * 512]
                elif mt < 4:
                    dest = kqv[f"yq{s}"][:, (mt - 2) * 1024:(mt - 2) * 1024 + 512]
                else:
                    dest = kqv[f"yq{s}"][:, (mt - 4) * 1024 + 512:(mt - 4) * 1024 + 1024]
            else:
                s, pair = divmod(pidx - 12, 2)
                for w4 in (2 * pair, 2 * pair + 1):
                    for kk in (0, 1):
                        nc.tensor.matmul(
                            ps[:, (w4 - 2 * pair) * 256:(w4 - 2 * pair + 1) * 256],
                            XT[s][:, kk * 512 + w4 * 128: kk * 512 + (w4 + 1) * 128],
                            wv[s][:, kk * 256:(kk + 1) * 256],
                            start=(kk == 0), stop=(kk == 1),
                        )
                dest = kqv[f"v{s}"][:, pair * 512:(pair + 1) * 512]
            if dr_scale is not None and dr_scale != 1.0:
                if evict_dve:
                    nc.vector.tensor_scalar_mul(dest, ps[:], dr_scale)
                else:
                    nc.scalar.activation(dest, ps[:], AF.Copy, scale=dr_scale)
            elif evict_dve:
                nc.vector.tensor_copy(dest, ps[:])
            else:
                nc.scalar.activation(dest, ps[:], AF.Copy)

        # ---- QK + exp + bias-mul for global wp index t ----
        def qk_stage(t, kqv):
            ci, w4 = divmod(t, CH)
            e0 = sb.tile([128, 2048], BF16, tag="e0", bufs=4, name=f"e0_{t}")
            qb = {}
            for j in range(4):
                qb[j] = pp.tile([128, 512], F32, tag=f"qkj{j}", name=f"qb{j}_{t}")
            # j innermost: consecutive matmuls hit different PE row groups
            # (tile rows j*32) so each LDWEIGHTS overlaps the in-flight matmul.
            for s in (0, 1):
                yk = kqv[f"yk{s}"]
                yq = kqv[f"yq{s}"]
                for hi in (0, 1):
                    for p in (0, 1):
                        for j in range(4):
                            kT = yk[j * 32:(j + 1) * 32,
                                    hi * 512 + w4 * 128 + p * 64:
                                    hi * 512 + w4 * 128 + p * 64 + 64]
                            q2 = yq[j * 32:(j + 1) * 32,
                                    hi * 1024:(hi + 1) * 1024].rearrange(
                                "p (r c) -> p r c", r=2)[
                                :, :, w4 * 128 + p * 64: w4 * 128 + p * 64 + 64]
                            nc.tensor.matmul(
                                qb[j][p * 64:p * 64 + 64,
                                      (2 * s + hi) * 128:(2 * s + hi + 1) * 128],
                                kT, q2, start=True, stop=True,
                                tile_position=(j * 32, p * 64),
                            )
            for j in range(4):
                nc.scalar.activation(e0[:, j * 512:(j + 1) * 512], qb[j][:],
                                     AF.Exp)
            E = sb.tile([128, 2048], BF16, tag="E", bufs=5, name=f"E{t}")
            # bias multiply split: GpSimd takes j01, DVE j23
            nc.gpsimd.tensor_mul(E[:, 0:1024], e0[:, 0:1024], ebt[:, 0:1024])
            nc.vector.tensor_mul(E[:, 1024:2048], e0[:, 1024:2048],
                                 ebt[:, 1024:2048])
            return E

        # ---- AV + sums + rec + mirror for wp t (part 1) ----
        # E col: j*512 + (2s+hi)*128 + t*64 + q
        def av_part1(t, E, kqv):
            ci, w4 = divmod(t, CH)
            o_b = {}
            d_b = {}
            for pp_ in (0, 1):
                o_b[pp_] = pp.tile([128, 512], F32, tag=f"o{pp_}",
                                   name=f"ob{pp_}_{t}")
                d_b[pp_] = pp.tile([128, 512], F32, tag=f"d{pp_}",
                                   name=f"db{pp_}_{t}")
            # pp_ innermost: consecutive matmuls alternate PE row groups
            # (tile rows pp_*64) so LDWEIGHTS overlaps the in-flight matmul.
            for s in (0, 1):
                for h in range(8):
                    hi, j = divmod(h, 4)
                    for pp_ in (0, 1):
                        ecols = E[pp_ * 64:pp_ * 64 + 64,
                                  j * 512 + (2 * s + hi) * 128:
                                  j * 512 + (2 * s + hi + 1) * 128]
                        vsl = kqv[f"v{s}"][pp_ * 64:pp_ * 64 + 64,
                                           w4 * 256 + h * 32:w4 * 256 + (h + 1) * 32]
                        cblk = (2 * s + hi) * 128
                        nc.tensor.matmul(
                            o_b[pp_][j * 32:(j + 1) * 32, cblk:cblk + 128],
                            vsl, ecols, start=True, stop=True,
                            tile_position=(pp_ * 64, j * 32),
                        )
            # hi-halves merged into one N=512 matmul per (j, pp_): the out AP
            # is reordered so the d_b column layout (hi, s, t, q) is kept.
            for j in range(4):
                for pp_ in (0, 1):
                    e2 = E[pp_ * 64:pp_ * 64 + 64, :].rearrange(
                        "q (jj ss c) -> q jj ss c", jj=4, ss=2)[:, j]
                    dout = d_b[pp_][j * 32:(j + 1) * 32, :].rearrange(
                        "p (hi ss c) -> p ss hi c", hi=2, ss=2)
                    nc.tensor.matmul(
                        dout, ones32[pp_ * 64:pp_ * 64 + 64, 0:32], e2,
                        start=True, stop=True,
                        tile_position=(pp_ * 64, j * 32),
                    )
            # mirror col = 512*s + 128*(2t+hi) + 64p + n
            mirror = sb.tile([128, 1024], BF16, tag="mir", bufs=3,
                             name=f"mb{t}")
            for pp_ in (0, 1):
                rec = sb.tile([128, 512], F32, tag=f"rec{pp_}", bufs=3,
                              name=f"rec{pp_}_{t}")
                nc.vector.reciprocal_approx_fast(rec[:], d_b[pp_][:])
                for s in (0, 1):
                    m_ap = mirror.rearrange(
                        "q (ss t hi pb n) -> q ss hi t pb n",
                        ss=2, t=2, hi=2, pb=2)[:, s, :, :, pp_, :]
                    nc.vector.tensor_mul(
                        m_ap,
                        o_b[pp_][:, 256 * s:256 * s + 256].rearrange(
                            "q (hi t n) -> q hi t n", hi=2, t=2),
                        rec.rearrange("q (hi ss t n) -> q ss hi t n",
                                      hi=2, ss=2, t=2)[:, s],
                    )
            return mirror

        # ---- out-projections + evict + DMA for wp t (part 2) ----
        def av_part2(t, mirror):
            # d1 (not qkj3): the qkj banks are on the hot QK/piece path, and a
            # proj2 write there makes the next wp's QK wait on MM completion.
            ps_o = pp.tile([128, 512], F32, tag="d1", name=f"op_{t}")
            for col, (nm, srcs, tt) in enumerate((
                ("x", (0, 0, 1, 1), 0),
                ("h", (1, 1, 0, 0), 1),
            )):
                for z in range(4):
                    s = srcs[z]
                    hi = z % 2
                    lhs = mirror[:, s * 512 + (2 * tt + hi) * 128:
                                 s * 512 + (2 * tt + hi + 1) * 128]
                    nc.tensor.matmul(
                        ps_o[:, col * 256:(col + 1) * 256],
                        lhs, wp_t[nm][:, z * 256:(z + 1) * 256],
                        start=(z == 0), stop=(z == 3),
                    )
            # batch output DMAs over wp pairs: halves the SP-queue dma_start
            # count (fixed DGE issue cost dominates these small transfers).
            if t % 4 == 0:
                osb_state["tile"] = sb.tile([128, 2048], BF16, tag="osb",
                                            bufs=2, name=f"osb{t}")
            o_sb = osb_state["tile"]
            nc.scalar.activation(o_sb[:, (t % 4) * 512:(t % 4) * 512 + 512],
                                 ps_o[:], AF.Copy)
            if t % 4 == 3:
                o2 = o_sb.rearrange("p (w c) -> p w c", w=4)
                for nm, dst in (("x", out_x), ("h", out_h)):
                    off = 0 if nm == "x" else 256
                    nc.sync.dma_start(
                        dst[(t - 3) * 128:(t + 1) * 128, :].rearrange(
                            "(w p) c -> p w c", w=4),
                        o2[:, :, off:off + 256])

        # ---- main pipeline ----
        XT_cur = dma_xt(0)
        kqv_cur = alloc_kqv(0)
        for pidx in range(16):
            piece(pidx, XT_cur, kqv_cur, evict_dve=(pidx % 4 < 2))
        XT_next = dma_xt(1)
        kqv_next = None

        # 2-deep E pipeline: AV consumes E two iterations behind so the
        # GpSimd bias-multiply (~5us) never blocks the PE.
        E_hist = {}
        kqv_hist = {}
        osb_state = {}

        def do_av(ta):
            mir = av_part1(ta, E_hist.pop(ta), kqv_hist.pop(ta))
            return mir

        for t in range(WP):
            ci, w4 = divmod(t, CH)
            if w4 == 0 and ci > 0:
                XT_cur, kqv_cur = XT_next, kqv_next
                if ci + 1 < NCH:
                    XT_next = dma_xt(ci + 1)
            if w4 == 0 and ci + 1 < NCH:
                kqv_next = alloc_kqv(ci + 1)

            # 1. AV + sums (+rec/mirror on DVE) for t-2
            mir = None
            if t >= 2 and "noav" not in KDBG:
                mir = do_av(t - 2)
            # 2. proj1/vproj pieces for chunk ci+1 (PE filler)
            if ci + 1 < NCH:
                for pidx in (4 * w4, 4 * w4 + 1, 4 * w4 + 2, 4 * w4 + 3):
                    piece(pidx, XT_next, kqv_next, evict_dve=(pidx % 4 < 2))
            # 3. QK + exp + bias-mul for t
            if "noqk" not in KDBG:
                E_hist[t] = qk_stage(t, kqv_cur)
                kqv_hist[t] = kqv_cur
            # 4. out-projection for t-2 (after DVE mirror)
            if mir is not None and "noproj2" not in KDBG:
                av_part2(t - 2, mir)

        # epilogue: drain last two wps
        if "noav" not in KDBG:
            for ta in (WP - 2, WP - 1):
                if ta in E_hist:
                    mir = av_part1(ta, E_hist.pop(ta), kqv_hist.pop(ta))
                    if "noproj2" not in KDBG:
                        av_part2(ta, mir)


def prep_inputs(inputs, n_cores=8):
    import ml_dtypes
    bf = ml_dtypes.bfloat16
    DIM_, HEADS, WS = 256, 8, 8
    N = WS * WS
    B = inputs["input_x"].shape[0]
    Wc = B // n_cores
    T = Wc * N
    hd = DIM_ // HEADS
    scale = hd ** -0.5

    Ws_, We_ = np.asarray(inputs["Ws"]), np.asarray(inputs["We"])
    bs_, be_ = np.asarray(inputs["bs"]), np.asarray(inputs["be"])
    assert np.all(bs_ == 0) and np.all(be_ == 0), "nonzero proj1 bias unsupported"
    Wpv_, Wph_ = np.asarray(inputs["Wpv"]), np.asarray(inputs["Wph"])
    rpi = np.asarray(inputs["rpi"])

    f8 = ml_dtypes.float8_e4m3fn

    def kq_weights(Wfull, s_v, s_h, dtype=bf):
        k = Wfull[:, 0:256]
        qv = Wfull[:, 512:768] * s_v
        qh = Wfull[:, 768:1024] * s_h
        w = np.concatenate([k, qv, qh], axis=1)
        return np.ascontiguousarray(w.reshape(2, 128, 768)).astype(dtype)

    wkq_s = kq_weights(Ws_, scale, scale * scale)
    wkq_e = kq_weights(We_, 1.0, scale)
    # fp8 DoubleRow weights are UNSCALED (scale applied at evacuation):
    # the scaled q-weights would underflow e4m3's subnormal range.
    wkq8_s = kq_weights(Ws_, 1.0, 1.0, f8)
    wkq8_e = np.ascontiguousarray(
        We_[:, 768:1024].reshape(2, 128, 256)).astype(f8)
    wv_s = np.ascontiguousarray(Ws_[:, 256:512].reshape(2, 128, 256)).astype(bf)
    wv_e = np.ascontiguousarray(We_[:, 256:512].reshape(2, 128, 256)).astype(bf)
    wpv = np.ascontiguousarray(Wpv_.reshape(4, 128, 256)).astype(bf)
    wph = np.ascontiguousarray(Wph_.reshape(4, 128, 256)).astype(bf)

    # exp(bias) master, j-major: col = j*512 + (2s+hi)*128 + t*64 + q
    tabs = {
        (0, 0): np.asarray(inputs["tcv"]), (0, 1): np.asarray(inputs["tsh"]),
        (1, 0): np.asarray(inputs["tsv"]), (1, 1): np.asarray(inputs["tch"]),
    }
    m = np.zeros((128, 2048), np.float32)
    for h in range(8):
        hi, j = divmod(h, 4)
        for s in (0, 1):
            for t, tab in ((0, tabs[(s, 0)]), (1, tabs[(s, 1)])):
                b = tab[rpi.reshape(-1), h].reshape(N, N)   # b[q, k]
                bt = np.exp(b.T)                             # [k, q]
                for p in (0, 1):
                    m[64 * p:64 * p + 64,
                      j * 512 + (2 * s + hi) * 128 + t * 64:
                      j * 512 + (2 * s + hi) * 128 + t * 64 + 64] = bt
    ebc = m.astype(bf)

    xs_all = np.asarray(inputs["state_x"], dtype=np.float32).reshape(n_cores, T, DIM_)
    xe_all = np.asarray(inputs["input_x"], dtype=np.float32).reshape(n_cores, T, DIM_)

    def xt_tiles(x, dtype=bf):
        xt = np.ascontiguousarray(x.T).astype(dtype)
        # [2, 128, NCH, 512] -> [NCH, 2, 128, 512]: contiguous per partition
        return np.ascontiguousarray(
            xt.reshape(2, 128, T // 512, 512).transpose(2, 0, 1, 3))

    in_maps = []
    for c in range(n_cores):
        m = {
            "xt_s": xt_tiles(xs_all[c]), "xt_e": xt_tiles(xe_all[c]),
            "wkq_s": wkq_s, "wkq_e": wkq_e,
            "wv_s": wv_s, "wv_e": wv_e,
            "wpv": wpv, "wph": wph, "ebc": ebc,
        }
        if USE_DR:
            m.update({
                "xt8_s": xt_tiles(xs_all[c], f8),
                "xt8_e": xt_tiles(xe_all[c], f8),
                "wkq8_s": wkq8_s, "wkq8_e": wkq8_e,
            })
        in_maps.append(m)
    return in_maps


def _get_compiled():
    key = W_CORE
    if key not in _cache:
        nc = bacc.Bacc("TRN2", target_bir_lowering=False, debug=False,
                       num_devices=N_CORES)
        with tile.TileContext(nc) as tc:
            build_kernel(nc, tc, W_CORE)
        nc.compile()
        _cache[key] = nc
    return _cache[key]


def kernel(**inputs):
    nc = _get_compiled()
    in_maps = prep_inputs(inputs, N_CORES)
    res = run_bass_kernel_spmd(nc, in_maps, list(range(N_CORES)), trace=False)
    bpv = np.asarray(inputs["bpv"])
    bph = np.asarray(inputs["bph"])
    B = np.asarray(inputs["input_x"]).shape[0]
    ox = np.concatenate([np.asarray(r["out_x"], dtype=np.float32)
                         for r in res.results], axis=0)
    oh = np.concatenate([np.asarray(r["out_h"], dtype=np.float32)
                         for r in res.results], axis=0)
    ox = (ox.reshape(B, NTOK, DIM) + bpv).astype(np.float32)
    oh = (oh.reshape(B, NTOK, DIM) + bph).astype(np.float32)
    return ox, oh



# revision 45
# speedup vs baseline: 1.0644x; 1.0644x over previous
"""Trainium2 Bass kernel for nn_BlockRecurrentSwinIRBlock (self-contained).

kernel(**inputs) takes the FULL unsharded inputs (B=2048 windows, 64 tokens,
C=256) and returns (output_x, state_out), each (2048, 64, 256) fp32.

Data-parallel over windows across 8 NeuronCores (256 windows/core).

v3 design: flat software-pipelined loop over window-pairs (wp = 128 tokens).
  - QK^T in 4 j-pure PSUM banks (one per 32-lane row group; mixed
    tile_position rows must never share a PSUM bank). E is j-major:
    col = j*512 + (2s+hi)*128 + t*64 + q, so every evacuation is contiguous.
  - exp on ScalarE (4 contiguous [128,512] ops), bias multiply
    E = exp(S)*exp(B) on the otherwise-idle GpSimd engine.
  - static per-purpose PSUM tags (8 banks exactly): qkj0..3 (QK banks,
    shared with 16 proj1/vproj piece tiles per chunk) + o0/o1/d0/d1
    (AV out + softmax sums; the merged out-projection bank shares d0).
  - proj1/vproj for chunk c+1 computed as PE filler interleaved into chunk
    c's attention iterations; fully unrolled loop, bf16 outputs.
"""
import os
import sys
import numpy as np

KDBG = set(os.environ.get("KDBG", "").split(","))

for _p in ("/opt/trn_rl_repo", "/opt/trn_rl_repo/concourse"):
    if _p not in sys.path:
        sys.path.insert(0, _p)

import concourse.bass as bass
import concourse.tile as tile
from concourse import bacc, mybir
from concourse.bass_utils import run_bass_kernel_spmd
from contextlib import ExitStack

N_CORES = 8
NTOK = 64
DIM = 256
B_TOTAL = 2048
W_CORE = B_TOTAL // N_CORES
CH = 4          # window-pairs per chunk

_cache = {}

BF16 = mybir.dt.bfloat16
F8 = mybir.dt.float8e4
F32 = mybir.dt.float32
AF = mybir.ActivationFunctionType
DR = mybir.MatmulPerfMode.DoubleRow
SCALE = (DIM // 8) ** -0.5   # hd^-0.5 = 0.17678
USE_DR = os.environ.get("KDR", "0") == "1"


def build_kernel(nc, tc, W):
    WP = W // 2          # window-pairs per core (128 tokens each)
    T = W * 64
    NCH = WP // CH
    assert WP % CH == 0

    # chunk-major, per-partition-contiguous: one 1KB run per partition per
    # DMA (128 descriptors) instead of the old per-wp strided gather (512).
    xt_s = nc.dram_tensor("xt_s", [WP // CH, 2, 128, CH * 128], BF16,
                          kind="ExternalInput").ap()
    xt_e = nc.dram_tensor("xt_e", [WP // CH, 2, 128, CH * 128], BF16,
                          kind="ExternalInput").ap()
    wkq_s = nc.dram_tensor("wkq_s", [2, 128, 768], BF16, kind="ExternalInput").ap()
    wkq_e = nc.dram_tensor("wkq_e", [2, 128, 768], BF16, kind="ExternalInput").ap()
    if USE_DR:
        xt8_s = nc.dram_tensor("xt8_s", [WP // CH, 2, 128, CH * 128], F8,
                               kind="ExternalInput").ap()
        xt8_e = nc.dram_tensor("xt8_e", [WP // CH, 2, 128, CH * 128], F8,
                               kind="ExternalInput").ap()
        # unscaled fp8 kq weights: state = full k|qv|qh, e = qh only
        wkq8_s = nc.dram_tensor("wkq8_s", [2, 128, 768], F8,
                                kind="ExternalInput").ap()
        wkq8_e = nc.dram_tensor("wkq8_e", [2, 128, 256], F8,
                                kind="ExternalInput").ap()
    wv_s = nc.dram_tensor("wv_s", [2, 128, 256], BF16, kind="ExternalInput").ap()
    wv_e = nc.dram_tensor("wv_e", [2, 128, 256], BF16, kind="ExternalInput").ap()
    wpv = nc.dram_tensor("wpv", [4, 128, 256], BF16, kind="ExternalInput").ap()
    wph = nc.dram_tensor("wph", [4, 128, 256], BF16, kind="ExternalInput").ap()
    # exp(bias) master, j-major: col = j*512 + (2s+hi)*128 + t*64 + q
    ebc = nc.dram_tensor("ebc", [128, 2048], BF16, kind="ExternalInput").ap()

    out_x = nc.dram_tensor("out_x", [T, 256], BF16, kind="ExternalOutput").ap()
    out_h = nc.dram_tensor("out_h", [T, 256], BF16, kind="ExternalOutput").ap()

    xt_d = {0: xt_s, 1: xt_e}
    xt8_d = {0: xt8_s, 1: xt8_e} if USE_DR else {}
    wv_d = {0: wv_s, 1: wv_e}

    with ExitStack() as ctx:
        wpool = ctx.enter_context(tc.tile_pool(name="weights", bufs=1))
        sb = ctx.enter_context(tc.tile_pool(name="sb", bufs=2))
        pp = ctx.enter_context(tc.tile_pool(name="pp", bufs=1, space="PSUM"))

        # ---- weights / constants (resident) ----
        wv = {}
        wkq = {}
        for s, apw in ((0, wkq_s), (1, wkq_e)):
            t_ = wpool.tile([128, 2 * 768], BF16, tag=f"wkq{s}", name=f"wkq{s}t")
            for kk in (0, 1):
                nc.sync.dma_start(t_[:, kk * 768:(kk + 1) * 768], apw[kk])
            wkq[s] = t_
        wkq8 = {}
        if USE_DR:
            w8s = wpool.tile([128, 2 * 768], F8, tag="wkq8s", name="wkq8st")
            for kk in (0, 1):
                nc.sync.dma_start(w8s[:, kk * 768:(kk + 1) * 768], wkq8_s[kk])
            w8e = wpool.tile([128, 2 * 256], F8, tag="wkq8e", name="wkq8et")
            for kk in (0, 1):
                nc.sync.dma_start(w8e[:, kk * 256:(kk + 1) * 256], wkq8_e[kk])
            wkq8 = {0: w8s, 1: w8e}
        for s in (0, 1):
            tv = wpool.tile([128, 2 * 256], BF16, tag=f"wv{s}", name=f"wv{s}t")
            for kk in (0, 1):
                nc.sync.dma_start(tv[:, kk * 256:(kk + 1) * 256], wv_d[s][kk])
            wv[s] = tv
        ebt = wpool.tile([128, 2048], BF16, tag="ebc", name="ebt")
        nc.sync.dma_start(ebt[:], ebc[:])
        wp_t = {}
        for nm, apw in (("x", wpv), ("h", wph)):
            t_ = wpool.tile([128, 4 * 256], BF16, tag=f"wp{nm}", name=f"wp{nm}t")
            for z in range(4):
                nc.sync.dma_start(t_[:, z * 256:(z + 1) * 256], apw[z])
            wp_t[nm] = t_
        ones32 = wpool.tile([128, 32], BF16, tag="ones", name="ones32")
        nc.vector.memset(ones32[:], 1.0)

        # ---- DMA of a chunk's XT (both sides, merged per kk) ----
        def dma_xt(ci):
            XT = {}
            for s in (0, 1):
                t_ = sb.tile([128, 1024], BF16, tag=f"xt{s}", bufs=2,
                             name=f"xt{s}_{ci}")
                for kk in (0, 1):
                    nc.sync.dma_start(t_[:, kk * 512:(kk + 1) * 512],
                                      xt_d[s][ci, kk])
                XT[s] = t_
                if USE_DR:
                    t8 = sb.tile([128, 1024], F8, tag=f"xt8{s}", bufs=2,
                                 name=f"xt8{s}_{ci}")
                    for kk in (0, 1):
                        nc.sync.dma_start(t8[:, kk * 512:(kk + 1) * 512],
                                          xt8_d[s][ci, kk])
                    XT[f"f8_{s}"] = t8
            return XT

        # ---- allocate the kqv tile set for one chunk ----
        def alloc_kqv(ci):
            kqv = {}
            for s in (0, 1):
                kqv[f"yk{s}"] = sb.tile([128, 1024], BF16, tag=f"yk{s}",
                                        bufs=3, name=f"yk{s}_{ci}")
                kqv[f"yq{s}"] = sb.tile([128, 2048], BF16, tag=f"yq{s}",
                                        bufs=3, name=f"yq{s}_{ci}")
                kqv[f"v{s}"] = sb.tile([128, 1024], BF16, tag=f"v{s}",
                                       bufs=3, name=f"v{s}_{ci}")
            return kqv

        # ---- one proj1/vproj piece for chunk ci (16 pieces: 0..15) ----
        # pieces 0..11: proj1 (s, mt); 12..15: vproj (s, wpair)
        # piece p uses psum tag qkj{p%4} ([128,512], 1 bank)
        def piece(pidx, XT, kqv, evict_dve):
            ps = pp.tile([128, 512], F32, tag=f"qkj{pidx % 4}",
                         name=f"pc{pidx}")
            dr_scale = None
            if pidx < 12:
                s, mt = divmod(pidx, 6)
                if USE_DR and (s == 0 or mt >= 4):
                    # fp8 DoubleRow piece with unscaled weights; the softmax
                    # scale is applied at PSUM evacuation.
                    dr_scale = 1.0
                    if mt >= 4:
                        dr_scale = SCALE * SCALE if s == 0 else SCALE
                    elif mt >= 2:
                        dr_scale = SCALE
                    w8 = wkq8[s]
                    mof = mt * 128 if s == 0 else (mt - 4) * 128
                    lhsT = w8.rearrange("p (kk c) -> p kk c", kk=2)[
                        :, :, mof:mof + 128]
                    rhs = XT[f"f8_{s}"].rearrange("p (kk n) -> p kk n", kk=2)
                    nc.tensor.matmul(ps[:], lhsT, rhs, start=True, stop=True,
                                     perf_mode=DR)
                else:
                    for kk in (0, 1):
                        nc.tensor.matmul(
                            ps[:],
                            wkq[s][:, kk * 768 + mt * 128: kk * 768 + (mt + 1) * 128],
                            XT[s][:, kk * 512:(kk + 1) * 512],
                            start=(kk == 0), stop=(kk == 1),
                        )
                if mt < 2:
                    dest = kqv[f"yk{s}"][:, mt * 512:(mt + 1) * 512]
                elif mt < 4:
                    dest = kqv[f"yq{s}"][:, (mt - 2) * 1024:(mt - 2) * 1024 + 512]
                else:
                    dest = kqv[f"yq{s}"][:, (mt - 4) * 1024 + 512:(mt - 4) * 1024 + 1024]
            else:
                s, pair = divmod(pidx - 12, 2)
                for w4 in (2 * pair, 2 * pair + 1):
                    for kk in (0, 1):
                        nc.tensor.matmul(
                            ps[:, (w4 - 2 * pair) * 256:(w4 - 2 * pair + 1) * 256],
                            XT[s][:, kk * 512 + w4 * 128: kk * 512 + (w4 + 1) * 128],
                            wv[s][:, kk * 256:(kk + 1) * 256],
                            start=(kk == 0), stop=(kk == 1),
                        )
                dest = kqv[f"v{s}"][:, pair * 512:(pair + 1) * 512]
            if dr_scale is not None and dr_scale != 1.0:
                if evict_dve:
                    nc.vector.tensor_scalar_mul(dest, ps[:], dr_scale)
                else:
                    nc.scalar.activation(dest, ps[:], AF.Copy, scale=dr_scale)
            elif evict_dve:
                nc.vector.tensor_copy(dest, ps[:])
            else:
                nc.scalar.activation(dest, ps[:], AF.Copy)

        # ---- QK + exp + bias-mul for global wp index t ----
        def qk_stage(t, kqv):
            ci, w4 = divmod(t, CH)
            e0 = sb.tile([128, 2048], BF16, tag="e0", bufs=4, name=f"e0_{t}")
            qb = {}
            for j in range(4):
                qb[j] = pp.tile([128, 512], F32, tag=f"qkj{j}", name=f"qb{j}_{t}")
            # j innermost: consecutive matmuls hit different PE row groups
            # (tile rows j*32) so each LDWEIGHTS overlaps the in-flight matmul.
            for s in (0, 1):
                yk = kqv[f"yk{s}"]
                yq = kqv[f"yq{s}"]
                for hi in (0, 1):
                    for p in (0, 1):
                        for j in range(4):
                            kT = yk[j * 32:(j + 1) * 32,
                                    hi * 512 + w4 * 128 + p * 64:
                                    hi * 512 + w4 * 128 + p * 64 + 64]
                            q2 = yq[j * 32:(j + 1) * 32,
                                    hi * 1024:(hi + 1) * 1024].rearrange(
                                "p (r c) -> p r c", r=2)[
                                :, :, w4 * 128 + p * 64: w4 * 128 + p * 64 + 64]
                            nc.tensor.matmul(
                                qb[j][p * 64:p * 64 + 64,
                                      (2 * s + hi) * 128:(2 * s + hi + 1) * 128],
                                kT, q2, start=True, stop=True,
                                tile_position=(j * 32, p * 64),
                            )
            E = sb.tile([128, 2048], BF16, tag="E", bufs=5, name=f"E{t}")
            # bias multiply per-j right after its exp, alternating DVE/GpSimd:
            # AV consumes j0/j1 columns first, so early blocks finish earliest
            # and neither engine waits on more exps than necessary.
            for j in range(4):
                nc.scalar.activation(e0[:, j * 512:(j + 1) * 512], qb[j][:],
                                     AF.Exp)
                eng = nc.vector if j % 2 == 0 else nc.gpsimd
                eng.tensor_mul(E[:, j * 512:(j + 1) * 512],
                               e0[:, j * 512:(j + 1) * 512],
                               ebt[:, j * 512:(j + 1) * 512])
            return E

        # ---- AV + sums + rec + mirror for wp t (part 1) ----
        # E col: j*512 + (2s+hi)*128 + t*64 + q
        def av_part1(t, E, kqv):
            ci, w4 = divmod(t, CH)
            o_b = {}
            d_b = {}
            for pp_ in (0, 1):
                o_b[pp_] = pp.tile([128, 512], F32, tag=f"o{pp_}",
                                   name=f"ob{pp_}_{t}")
                d_b[pp_] = pp.tile([128, 512], F32, tag=f"d{pp_}",
                                   name=f"db{pp_}_{t}")
            # pp_ innermost: consecutive matmuls alternate PE row groups
            # (tile rows pp_*64) so LDWEIGHTS overlaps the in-flight matmul.
            for s in (0, 1):
                for h in range(8):
                    hi, j = divmod(h, 4)
                    for pp_ in (0, 1):
                        ecols = E[pp_ * 64:pp_ * 64 + 64,
                                  j * 512 + (2 * s + hi) * 128:
                                  j * 512 + (2 * s + hi + 1) * 128]
                        vsl = kqv[f"v{s}"][pp_ * 64:pp_ * 64 + 64,
                                           w4 * 256 + h * 32:w4 * 256 + (h + 1) * 32]
                        cblk = (2 * s + hi) * 128
                        nc.tensor.matmul(
                            o_b[pp_][j * 32:(j + 1) * 32, cblk:cblk + 128],
                            vsl, ecols, start=True, stop=True,
                            tile_position=(pp_ * 64, j * 32),
                        )
            # hi-halves merged into one N=512 matmul per (j, pp_): the out AP
            # is reordered so the d_b column layout (hi, s, t, q) is kept.
            for j in range(4):
                for pp_ in (0, 1):
                    e2 = E[pp_ * 64:pp_ * 64 + 64, :].rearrange(
                        "q (jj ss c) -> q jj ss c", jj=4, ss=2)[:, j]
                    dout = d_b[pp_][j * 32:(j + 1) * 32, :].rearrange(
                        "p (hi ss c) -> p ss hi c", hi=2, ss=2)
                    nc.tensor.matmul(
                        dout, ones32[pp_ * 64:pp_ * 64 + 64, 0:32], e2,
                        start=True, stop=True,
                        tile_position=(pp_ * 64, j * 32),
                    )
            # mirror col = 512*s + 128*(2t+hi) + 64p + n
            mirror = sb.tile([128, 1024], BF16, tag="mir", bufs=3,
                             name=f"mb{t}")
            for pp_ in (0, 1):
                rec = sb.tile([128, 512], F32, tag=f"rec{pp_}", bufs=3,
                              name=f"rec{pp_}_{t}")
                nc.vector.reciprocal_approx_fast(rec[:], d_b[pp_][:])
                for s in (0, 1):
                    m_ap = mirror.rearrange(
                        "q (ss t hi pb n) -> q ss hi t pb n",
                        ss=2, t=2, hi=2, pb=2)[:, s, :, :, pp_, :]
                    nc.vector.tensor_mul(
                        m_ap,
                        o_b[pp_][:, 256 * s:256 * s + 256].rearrange(
                            "q (hi t n) -> q hi t n", hi=2, t=2),
                        rec.rearrange("q (hi ss t n) -> q ss hi t n",
                                      hi=2, ss=2, t=2)[:, s],
                    )
            return mirror

        # ---- out-projections + evict + DMA for wp t (part 2) ----
        def av_part2(t, mirror):
            # d1 (not qkj3): the qkj banks are on the hot QK/piece path, and a
            # proj2 write there makes the next wp's QK wait on MM completion.
            ps_o = pp.tile([128, 512], F32, tag="d1", name=f"op_{t}")
            for col, (nm, srcs, tt) in enumerate((
                ("x", (0, 0, 1, 1), 0),
                ("h", (1, 1, 0, 0), 1),
            )):
                for z in range(4):
                    s = srcs[z]
                    hi = z % 2
                    lhs = mirror[:, s * 512 + (2 * tt + hi) * 128:
                                 s * 512 + (2 * tt + hi + 1) * 128]
                    nc.tensor.matmul(
                        ps_o[:, col * 256:(col + 1) * 256],
                        lhs, wp_t[nm][:, z * 256:(z + 1) * 256],
                        start=(z == 0), stop=(z == 3),
                    )
            # batch output DMAs over wp pairs: halves the SP-queue dma_start
            # count (fixed DGE issue cost dominates these small transfers).
            if t % 2 == 0:
                osb_state["tile"] = sb.tile([128, 1024], BF16, tag="osb",
                                            bufs=2, name=f"osb{t}")
            o_sb = osb_state["tile"]
            nc.scalar.activation(o_sb[:, (t % 2) * 512:(t % 2) * 512 + 512],
                                 ps_o[:], AF.Copy)
            if t % 2 == 1:
                o2 = o_sb.rearrange("p (w c) -> p w c", w=2)
                for nm, dst in (("x", out_x), ("h", out_h)):
                    off = 0 if nm == "x" else 256
                    nc.sync.dma_start(
                        dst[(t - 1) * 128:(t + 1) * 128, :].rearrange(
                            "(w p) c -> p w c", w=2),
                        o2[:, :, off:off + 256])

        # ---- main pipeline ----
        XT_cur = dma_xt(0)
        kqv_cur = alloc_kqv(0)
        for pidx in range(16):
            piece(pidx, XT_cur, kqv_cur, evict_dve=(pidx % 4 < 2))
        XT_next = dma_xt(1)
        kqv_next = None

        # 2-deep E pipeline: AV consumes E two iterations behind so the
        # GpSimd bias-multiply (~5us) never blocks the PE.
        E_hist = {}
        kqv_hist = {}
        osb_state = {}

        def do_av(ta):
            mir = av_part1(ta, E_hist.pop(ta), kqv_hist.pop(ta))
            return mir

        for t in range(WP):
            ci, w4 = divmod(t, CH)
            if w4 == 0 and ci > 0:
                XT_cur, kqv_cur = XT_next, kqv_next
                if ci + 1 < NCH:
                    XT_next = dma_xt(ci + 1)
            if w4 == 0 and ci + 1 < NCH:
                kqv_next = alloc_kqv(ci + 1)

            # 1. AV + sums (+rec/mirror on DVE) for t-2
            mir = None
            if t >= 2 and "noav" not in KDBG:
                mir = do_av(t - 2)
            # 2. proj1/vproj pieces for chunk ci+1 (PE filler)
            if ci + 1 < NCH:
                for pidx in (4 * w4, 4 * w4 + 1, 4 * w4 + 2, 4 * w4 + 3):
                    piece(pidx, XT_next, kqv_next, evict_dve=(pidx % 4 < 2))
            # 3. QK + exp + bias-mul for t
            if "noqk" not in KDBG:
                E_hist[t] = qk_stage(t, kqv_cur)
                kqv_hist[t] = kqv_cur
            # 4. out-projection for t-2 (after DVE mirror)
            if mir is not None and "noproj2" not in KDBG:
                av_part2(t - 2, mir)

        # epilogue: drain last two wps
        if "noav" not in KDBG:
            for ta in (WP - 2, WP - 1):
                if ta in E_hist:
                    mir = av_part1(ta, E_hist.pop(ta), kqv_hist.pop(ta))
                    if "noproj2" not in KDBG:
                        av_part2(ta, mir)


def prep_inputs(inputs, n_cores=8):
    import ml_dtypes
    bf = ml_dtypes.bfloat16
    DIM_, HEADS, WS = 256, 8, 8
    N = WS * WS
    B = inputs["input_x"].shape[0]
    Wc = B // n_cores
    T = Wc * N
    hd = DIM_ // HEADS
    scale = hd ** -0.5

    Ws_, We_ = np.asarray(inputs["Ws"]), np.asarray(inputs["We"])
    bs_, be_ = np.asarray(inputs["bs"]), np.asarray(inputs["be"])
    assert np.all(bs_ == 0) and np.all(be_ == 0), "nonzero proj1 bias unsupported"
    Wpv_, Wph_ = np.asarray(inputs["Wpv"]), np.asarray(inputs["Wph"])
    rpi = np.asarray(inputs["rpi"])

    f8 = ml_dtypes.float8_e4m3fn

    def kq_weights(Wfull, s_v, s_h, dtype=bf):
        k = Wfull[:, 0:256]
        qv = Wfull[:, 512:768] * s_v
        qh = Wfull[:, 768:1024] * s_h
        w = np.concatenate([k, qv, qh], axis=1)
        return np.ascontiguousarray(w.reshape(2, 128, 768)).astype(dtype)

    wkq_s = kq_weights(Ws_, scale, scale * scale)
    wkq_e = kq_weights(We_, 1.0, scale)
    # fp8 DoubleRow weights are UNSCALED (scale applied at evacuation):
    # the scaled q-weights would underflow e4m3's subnormal range.
    wkq8_s = kq_weights(Ws_, 1.0, 1.0, f8)
    wkq8_e = np.ascontiguousarray(
        We_[:, 768:1024].reshape(2, 128, 256)).astype(f8)
    wv_s = np.ascontiguousarray(Ws_[:, 256:512].reshape(2, 128, 256)).astype(bf)
    wv_e = np.ascontiguousarray(We_[:, 256:512].reshape(2, 128, 256)).astype(bf)
    wpv = np.ascontiguousarray(Wpv_.reshape(4, 128, 256)).astype(bf)
    wph = np.ascontiguousarray(Wph_.reshape(4, 128, 256)).astype(bf)

    # exp(bias) master, j-major: col = j*512 + (2s+hi)*128 + t*64 + q
    tabs = {
        (0, 0): np.asarray(inputs["tcv"]), (0, 1): np.asarray(inputs["tsh"]),
        (1, 0): np.asarray(inputs["tsv"]), (1, 1): np.asarray(inputs["tch"]),
    }
    m = np.zeros((128, 2048), np.float32)
    for h in range(8):
        hi, j = divmod(h, 4)
        for s in (0, 1):
            for t, tab in ((0, tabs[(s, 0)]), (1, tabs[(s, 1)])):
                b = tab[rpi.reshape(-1), h].reshape(N, N)   # b[q, k]
                bt = np.exp(b.T)                             # [k, q]
                for p in (0, 1):
                    m[64 * p:64 * p + 64,
                      j * 512 + (2 * s + hi) * 128 + t * 64:
                      j * 512 + (2 * s + hi) * 128 + t * 64 + 64] = bt
    ebc = m.astype(bf)

    xs_all = np.asarray(inputs["state_x"], dtype=np.float32).reshape(n_cores, T, DIM_)
    xe_all = np.asarray(inputs["input_x"], dtype=np.float32).reshape(n_cores, T, DIM_)

    def xt_tiles(x, dtype=bf):
        xt = np.ascontiguousarray(x.T).astype(dtype)
        # [2, 128, NCH, 512] -> [NCH, 2, 128, 512]: contiguous per partition
        return np.ascontiguousarray(
            xt.reshape(2, 128, T // 512, 512).transpose(2, 0, 1, 3))

    in_maps = []
    for c in range(n_cores):
        m = {
            "xt_s": xt_tiles(xs_all[c]), "xt_e": xt_tiles(xe_all[c]),
            "wkq_s": wkq_s, "wkq_e": wkq_e,
            "wv_s": wv_s, "wv_e": wv_e,
            "wpv": wpv, "wph": wph, "ebc": ebc,
        }
        if USE_DR:
            m.update({
                "xt8_s": xt_tiles(xs_all[c], f8),
                "xt8_e": xt_tiles(xe_all[c], f8),
                "wkq8_s": wkq8_s, "wkq8_e": wkq8_e,
            })
        in_maps.append(m)
    return in_maps


def _get_compiled():
    key = W_CORE
    if key not in _cache:
        nc = bacc.Bacc("TRN2", target_bir_lowering=False, debug=False,
                       num_devices=N_CORES)
        with tile.TileContext(nc) as tc:
            build_kernel(nc, tc, W_CORE)
        nc.compile()
        _cache[key] = nc
    return _cache[key]


def kernel(**inputs):
    nc = _get_compiled()
    in_maps = prep_inputs(inputs, N_CORES)
    res = run_bass_kernel_spmd(nc, in_maps, list(range(N_CORES)), trace=False)
    bpv = np.asarray(inputs["bpv"])
    bph = np.asarray(inputs["bph"])
    B = np.asarray(inputs["input_x"]).shape[0]
    ox = np.concatenate([np.asarray(r["out_x"], dtype=np.float32)
                         for r in res.results], axis=0)
    oh = np.concatenate([np.asarray(r["out_h"], dtype=np.float32)
                         for r in res.results], axis=0)
    ox = (ox.reshape(B, NTOK, DIM) + bpv).astype(np.float32)
    oh = (oh.reshape(B, NTOK, DIM) + bph).astype(np.float32)
    return ox, oh

